# revision 17
# baseline (speedup 1.0000x reference)
"""Pairwise Euclidean distance matrix on 8 TRN2 NeuronCores (Bass/Tile).

out[i, j] = ||x[j] - x[i]||_2 for x [4096, 512] fp32.

Distance symmetry: out = out.T. Core c owns query block c and computes
it against key blocks {c, c+1, c+2, c+3 mod 8} in full, plus a HALF
share of the ring-distance-4 pair {p, p+4} (p = c mod 4): keys block
p+4 against 256 queries of block p (first half on cores 0-3, second on
4-7). Every unordered block pair is computed exactly once and mirrored
on the host — no duplicated work, perfectly balanced, SPMD-uniform.

Layout: queries on PSUM partitions, keys on the free axis. The Gram
part runs as fp8 e4m3 DoubleRow matmuls (2 fp8 weights/cell, 2 MACs/
cycle); -2 is pre-folded into the quantized queries. A tiny
[2,128]x[2,512] bf16 augmentation matmul per PSUM tile adds sq_m
(per-query) + sq_n (per-key) into the same accumulation, so PSUM holds
d^2 directly. Epilogue per supertile half is one big ACT Sqrt
(PSUM -> SBUF bf16) and one output DMA in SBUF-native layout (host
unscrambles). Quantization error lands ~7e-3 on the harness metric
(gate 2e-2). Diagonal d^2 can go slightly negative under fp8 -> NaN
after sqrt; the host overwrites the diagonal.
"""

import numpy as np
import ml_dtypes

import concourse.bass as bass
import concourse.bacc as bacc
import concourse.tile as tile
from concourse.bass_utils import run_bass_kernel_spmd

mybir = bass.mybir

N = 4096          # number of points
D = 512           # feature dim
NCORES = 8
QB = N // NCORES  # 512 queries per core
NC = 5            # key chunks of 512 per core (last one half-query)
KEYS = NC * 512   # 2560 keys per core
NS = 4            # query subblocks of 128
HQ = 256          # queries in the half chunk

_FP8 = mybir.dt.float8e4
_BF16 = mybir.dt.bfloat16
_F32 = mybir.dt.float32

_nc_cache = {}


def _build():
    if "nc" in _nc_cache:
        return _nc_cache["nc"]
    nc = bacc.Bacc("TRN2", target_bir_lowering=False, debug=False)

    # keys: [128, chunk, k-subtile, 512] e4m3 packed host-side so each
    # chunk DMA reads 2048B contiguous per partition
    xp = nc.dram_tensor("xp", [128, NC * 4 * 512], _FP8, kind="ExternalInput")
    # own queries [128, k-subtile, 512] e4m3, pre-scaled by -2
    q = nc.dram_tensor("q", [128, 4 * QB], _FP8, kind="ExternalInput")
    # foreign half-block queries for the distance-4 chunk
    q2 = nc.dram_tensor("q2", [128, 4 * HQ], _FP8, kind="ExternalInput")
    # augmentation rows (bf16): cols 0:QB = (own sq_m; ones),
    # QB:QB+HQ = (foreign sq_m; ones), then (ones; sq_n) per key chunk
    aug = nc.dram_tensor("aug", [2, QB + HQ + KEYS], _BF16, kind="ExternalInput")
    # output in SBUF-native layout: 4 full chunks then the half chunk
    out = nc.dram_tensor("out", [128, 4 * 2048 + 1024], _BF16, kind="ExternalOutput")

    sqrt = mybir.ActivationFunctionType.Sqrt
    dr = mybir.MatmulPerfMode.DoubleRow
    AK = QB + HQ  # aug key-row base

    with tile.TileContext(nc) as tc:
        with (
            tc.tile_pool(name="xd", bufs=1) as xd,
            tc.tile_pool(name="op", bufs=3) as op,
            tc.tile_pool(name="ps", bufs=2, space="PSUM") as pp,
        ):
            # Inputs spread over the three trigger engines' DMA rings
            # (same-engine DMAs serialize on that ring, and each DMA has
            # ~2.5us trigger+completion latency): first-needed tensors
            # get their own ring. Flat 2048B-contiguous loads; matmul
            # slices come from strided AP views.
            t_k, kv = [], []
            for c in range(NC):
                t = xd.tile([128, 4 * 512], _FP8, tag=f"k{c}", name=f"k{c}")
                eng = [nc.sync, nc.scalar, nc.gpsimd, nc.sync, nc.scalar][c]
                eng.dma_start(t[:], xp.ap()[:, c * 2048 : (c + 1) * 2048])
                t_k.append(t)
                kv.append(t[:].rearrange("p (kp k n) -> p kp k n", kp=2, k=2))
                if c == 0:
                    t_q = xd.tile([128, 4 * QB], _FP8, tag="q", name="q")
                    nc.gpsimd.dma_start(t_q[:], q.ap())
                    qv = t_q[:].rearrange("p (kp k n) -> p kp k n", kp=2, k=2)
                    t_aug = xd.tile(
                        [2, QB + HQ + KEYS], _BF16, tag="aug", name="aug"
                    )
                    nc.gpsimd.dma_start(t_aug[:], aug.ap())
                if c == 1:
                    t_q2 = xd.tile([128, 4 * HQ], _FP8, tag="q2", name="q2")
                    nc.gpsimd.dma_start(t_q2[:], q2.ap())
                    qv2 = t_q2[:].rearrange(
                        "p (kp k n) -> p kp k n", kp=2, k=2
                    )

            # PE warmup, chained behind gpsimd's trigger queue (memset
            # emitted after its 3 DMA triggers, ~8.9us): the dummies span
            # the window until chunk-0's inputs are resident (~10.4us)
            # and flow gapless into the real matmuls. Starting the real
            # stream before inputs are staged makes chunks hit DMA-
            # completion waits mid-stream; those sub-us gaps make the HAM
            # clock gate oscillate between 1.2/2.4 GHz on a 2-window
            # period (measured +6us). A gapless stream warms once at
            # ~3.4us and stays warm.
            warm = xd.tile([128, QB], _BF16, tag="warm", name="warm")
            nc.gpsimd.memset(warm[:], 0.0)
            # ACT sqrt table preload (~2.7us) on a dedicated tile
            dumm = xd.tile([128, 1], _F32, tag="dumm", name="dumm")
            nc.vector.memset(dumm[:], 1.0)
            nc.scalar.activation(dumm[:], dumm[:], sqrt, bias=0.0, scale=1.0)
            wps = pp.tile([128, NS * 512], _F32, tag="ps", name="wps")
            for _ in range(6):
                nc.tensor.matmul(
                    wps[:, 0:QB], warm[:, 0:128], warm[:], start=True, stop=True
                )

            for c in range(4):
                psg = pp.tile([128, NS * 512], _F32, tag="ps", name=f"ps{c}")
                o = op.tile([128, NS * 512], _BF16, tag="o", name=f"o{c}")
                # kp-major: consecutive matmuls hit different PSUM banks
                # so fills pipeline (same-bank back-to-back accumulation
                # exposes the ~250-cycle drain).
                for kp in (0, 1):
                    for s in range(NS):
                        nc.tensor.matmul(
                            psg[:, s * 512 : (s + 1) * 512],
                            qv[:, kp, :, s * 128 : (s + 1) * 128],
                            kv[c][:, kp, :, :],
                            start=(kp == 0),
                            stop=False,
                            perf_mode=dr,
                        )
                for s in range(NS):
                    nc.tensor.matmul(
                        psg[:, s * 512 : (s + 1) * 512],
                        t_aug[:, s * 128 : (s + 1) * 128],
                        t_aug[:, AK + c * 512 : AK + (c + 1) * 512],
                        start=False,
                        stop=True,
                    )
                for h in (0, 1):
                    hl = slice(h * 1024, (h + 1) * 1024)
                    nc.scalar.activation(
                        o[:, hl], psg[:, hl], sqrt, bias=0.0, scale=1.0
                    )
                    dst = out.ap()[
                        :, c * 2048 + h * 1024 : c * 2048 + (h + 1) * 1024
                    ]
                    eng = nc.gpsimd if h == 0 else nc.sync
                    eng.dma_start(dst, o[:, hl])

            # half chunk: distance-4 pair, 256 foreign queries
            psg = pp.tile([128, 2 * 512], _F32, tag="ps", name="ps4")
            o = op.tile([128, 2 * 512], _BF16, tag="o", name="o4")
            for kp in (0, 1):
                for s in range(2):
                    nc.tensor.matmul(
                        psg[:, s * 512 : (s + 1) * 512],
                        qv2[:, kp, :, s * 128 : (s + 1) * 128],
                        kv[4][:, kp, :, :],
                        start=(kp == 0),
                        stop=False,
                        perf_mode=dr,
                    )
            for s in range(2):
                nc.tensor.matmul(
                    psg[:, s * 512 : (s + 1) * 512],
                    t_aug[:, QB + s * 128 : QB + (s + 1) * 128],
                    t_aug[:, AK + 4 * 512 : AK + 5 * 512],
                    start=False,
                    stop=True,
                )
            nc.scalar.activation(o[:], psg[:], sqrt, bias=0.0, scale=1.0)
            nc.sync.dma_start(out.ap()[:, 8192:9216], o[:])

    nc.compile()
    _nc_cache["nc"] = nc
    return nc


def _prep_inputs(x: np.ndarray):
    x = np.ascontiguousarray(x, dtype=np.float32)
    x8 = x.astype(ml_dtypes.float8_e4m3)       # keys [N, D]
    q8 = (-2.0 * x).astype(ml_dtypes.float8_e4m3)
    sqv = np.einsum("nd,nd->n", x.astype(np.float64), x.astype(np.float64))
    sqb = sqv.astype(ml_dtypes.bfloat16)
    ones = np.ones(N, dtype=ml_dtypes.bfloat16)

    def pack_q(rows):  # [n, D] -> [128, 4*n] (p, kp*k, n)
        n = rows.shape[0]
        return np.ascontiguousarray(
            rows.reshape(n, 4, 128).transpose(2, 1, 0).reshape(128, 4 * n)
        )

    in_maps = []
    for c in range(NCORES):
        p = c % 4
        half = slice(0, HQ) if c < 4 else slice(HQ, QB)
        r0 = c * QB
        kblocks = [(c + t) % NCORES for t in range(4)] + [p + 4]
        keycols = np.concatenate(
            [np.arange(r * QB, (r + 1) * QB) for r in kblocks]
        )
        # keys: [p, chunk, ksub, n] with feature 128*ksub+p
        kc = x8[keycols, :].reshape(NC, 512, 4, 128)  # [c, n, k, p]
        xp_pack = kc.transpose(3, 0, 2, 1).reshape(128, NC * 4 * 512)
        fq = np.arange(p * QB, (p + 1) * QB)[half]  # foreign query rows
        aug_pack = np.empty((2, QB + HQ + KEYS), dtype=ml_dtypes.bfloat16)
        aug_pack[0, 0:QB] = sqb[r0 : r0 + QB]
        aug_pack[1, 0:QB] = ones[0:QB]
        aug_pack[0, QB : QB + HQ] = sqb[fq]
        aug_pack[1, QB : QB + HQ] = ones[0:HQ]
        aug_pack[0, QB + HQ :] = ones[0:KEYS]
        aug_pack[1, QB + HQ :] = sqb[keycols]
        in_maps.append(
            {
                "xp": np.ascontiguousarray(xp_pack),
                "q": pack_q(q8[r0 : r0 + QB, :]),
                "q2": pack_q(q8[fq, :]),
                "aug": np.ascontiguousarray(aug_pack),
            }
        )
    return in_maps


def run(x: np.ndarray, trace: bool = False, tmpdir: str | None = None):
    nc = _build()
    in_maps = _prep_inputs(x)
    res = run_bass_kernel_spmd(
        nc, in_maps, list(range(NCORES)), trace=trace, tmpdir=tmpdir
    )
    full = np.empty((N, N), dtype=np.float32)
    for c in range(NCORES):
        p = c % 4
        h0 = 0 if c < 4 else HQ
        o = res.results[c]["out"].astype(np.float32)
        # full chunks: [p, c, s, n] -> blk[q = s*128+p, key = c*512+n]
        blk = (
            o[:, 0:8192].reshape(128, 4, NS, 512)
            .transpose(2, 0, 1, 3).reshape(QB, 4 * 512)
        )
        for t in range(4):
            r = (c + t) % NCORES
            b = blk[:, t * 512 : (t + 1) * 512]  # [queries c, keys r]
            full[r * QB : (r + 1) * QB, c * QB : (c + 1) * QB] = b.T
            if t > 0:
                full[c * QB : (c + 1) * QB, r * QB : (r + 1) * QB] = b
        # half chunk: [queries block p (half), keys block p+4]
        b4 = (
            o[:, 8192:9216].reshape(128, 2, 512)
            .transpose(1, 0, 2).reshape(HQ, 512)
        )
        r4 = (p + 4) * QB
        full[r4 : r4 + QB, p * QB + h0 : p * QB + h0 + HQ] = b4.T
        full[p * QB + h0 : p * QB + h0 + HQ, r4 : r4 + QB] = b4
    np.fill_diagonal(full, 0.0)
    return full, res


def kernel(x: np.ndarray) -> np.ndarray:
    out, _ = run(x, trace=False)
    return out
